# revision 11
# baseline (speedup 1.0000x reference)
"""Trainium2 Bass kernel for per-sample dynamic 3x3 conv (periodic padding).

y[b,o,h,w] = sum_{c,i,j} x[b,c,(h+i-1)%H,(w+j-1)%W] * wgt[b, c*9+i*3+j, o] + bias[b,o]

Shapes: x [16,64,128,128] f32, wgt [16,576,64] f32, bias [16,64] f32.
Sharding: data-parallel over batch, 2 samples per core on 8 cores.

Compute scheme: 64x64 PE-array tiling. Every matmul is K=64 (C), M=64 (O),
so four quadrant matmuls run concurrently on the 128x128 array — full
useful-MAC utilization (vs 50% for block-diagonal K=128 packing). The
quadrant grid over a spatial tile of 8 output rows:
  array rows (rhs stream): sample s lives in SBUF partitions 64s:64s+64
  array cols (PSUM rows):  col-group g computes output rows 4g:4g+4
so each quadrant (s,g) accumulates the FULL 9-tap sum for its own quarter
of the output — no cross-quadrant reduction. PSUM layout per spatial tile
is one [128, 2, 4, W] tile (2 banks; sample = bank, quadrant = partition
half x bank), every quadrant owning a private (partition x bank) region so
all four accumulation chains start/stop independently.

The tap loop runs OUTERMOST over a group of 2 spatial tiles so each
quadrant's stationary weights are loaded once per 2 matmuls (amortizing
LDWEIGHTS); PSUM holds 2 groups in flight (8 banks) so groups pipeline
without stalls.

Periodic padding needs no padded-image build: matmuls read the raw image;
the column wrap (j=0/2) splits each tap into a 1-wide and a 127-wide
piece, the row wrap splits the two boundary tiles (image row H-1 loads
first so tile 0 starts immediately).

Evacuation per sample: the g=s quadrant is already on the output channel
partitions -> DVE tensor_scalar_add(+bias) straight into the bf16 staging
tile; the g=1-s quadrant needs a partition-crossing copy -> ACT
activation(Identity, +bias), which the DVE crossbar/ACT support for
64-partition ops. Inputs/weights are cast to bf16 host-side and the
output is stored bf16 and upcast on the host: DMA traffic halves so the
~31us PE stream is the critical path (HBM ~424 GB/s aggregate).
"""

import numpy as np

KH = KW = 3
B, C, O, H, W = 16, 64, 64, 128, 128
N_CORES = 8
BPC = B // N_CORES  # samples per core
TILE_ROWS = 8  # output rows per spatial tile (4 per quadrant col-group)
QROWS = TILE_ROWS // 2  # rows per quadrant -> N = 4*128 = 512
N_TILES = H // TILE_ROWS
G = 2  # spatial tiles per tap-outer group
OGROUP = 4  # spatial tiles per output store group (32 rows)

TAPS = [(1, 1), (0, 1), (2, 1), (1, 0), (1, 2), (0, 0), (0, 2), (2, 0), (2, 2)]

_CACHE = {}


def _patch_tile_drain():
    """This container's walrus rejects Drain instructions carrying more than
    one sem wait (setupSyncWait: Too many sync wait commands). Re-emit the
    TileContext exit drain's waits as individual wait_ge instructions."""
    import concourse.tile as tile
    from concourse.vector_clock import ScopedClock

    if getattr(tile.TileContext, "_drain_patch_applied", False):
        return

    def _drain_and_barrier(self, tick_clock, wait_clock):
        import concourse.mybir as mybir

        nc = self.nc
        nop = nc.sync.nop(nofuse=True)
        wait_clock.add_sem_waits(nop.ins, ScopedClock({None: tick_clock.global_clock}))
        waits = list(nop.ins.sync_info.on_wait)
        nop.ins.sync_info.on_wait.clear()
        assert self.sems is not None
        by_name = {}
        for h in self.sems.allocated().values():
            by_name[getattr(h, "name", None)] = h
        # Spread the final sem waits round-robin over all engine queues
        # (serial ~115ns/wait on one queue otherwise); the sem-only barrier
        # below restores the all-engines ordering.
        engs = [nc.sync, nc.vector, nc.scalar, nc.tensor, nc.gpsimd]
        for k, w in enumerate(waits):
            h = by_name.get(w.ant_name)
            assert h is not None, f"no sem handle for {w.ant_name}"
            engs[k % len(engs)].wait_ge(h, w.wait_value)
        # per-engine drains except GpSimd's expensive dge_drain, then a
        # sem-only barrier (mirrors BassBlock.no_gpsimd_drain)
        gpsimd_type = nc.gpsimd.engine
        for eng_type, eng in nc.engines.items():
            if eng_type == gpsimd_type:
                continue
            d = mybir.InstDrain(
                name=nc.get_next_instruction_name(),
                ins=[],
                outs=[],
                bass_is_fusable=False,
            )
            d.engine = eng_type
            eng.add_instruction(d)
        nc.all_engine_barrier(sem_only=True)
        popped = nc._tile_sem_poison_stack.pop()
        assert popped is self._sem_poison
        nc.clear_and_free_semaphores(list(self.sems.allocated().values()))

    tile.TileContext._drain_and_barrier = _drain_and_barrier
    tile.TileContext._drain_patch_applied = True


def _split_multi_waits(nc, max_waits=1):
    """Same walrus limitation, general form: any instruction carrying more
    than one sem wait fails setupSyncWait. Hoist excess waits onto dedicated
    single-wait NOPs on the same engine, placed just before the instruction."""
    import concourse.mybir as mybir

    for f in nc.m.functions:
        for blk in f.blocks:
            out = []
            changed = False
            for inst in blk.instructions:
                si = getattr(inst, "sync_info", None)
                waits = list(si.on_wait) if si is not None else []
                if len(waits) > max_waits:
                    changed = True
                    for w in waits[:-max_waits]:
                        out.append(
                            mybir.InstNoOp(
                                name=nc.get_next_instruction_name(),
                                engine=inst.engine,
                                sync_info=mybir.SyncInfo(on_wait=[w], on_update=[]),
                                bass_nofuse=True,
                            )
                        )
                    si.on_wait.clear()
                    for w in waits[-max_waits:]:
                        si.on_wait.append(w)
                out.append(inst)
            if changed:
                blk.instructions = out


def _row_pieces(r0):
    """(out_row0, out_row1, img_row0) pieces for a quadrant block whose
    kernel-shifted image rows start at r0 (may wrap at either end)."""
    if r0 < 0:  # t=0, g=0, i=0: out row 0 reads image row H-1
        return [(0, 1, H - 1), (1, QROWS, 0)]
    if r0 + QROWS > H:  # t=last, g=1, i=2: last out row reads image row 0
        return [(0, QROWS - 1, r0), (QROWS - 1, QROWS, 0)]
    return [(0, QROWS, r0)]


def _col_pieces(j):
    """(out_col0, out_col1, img_col0) pieces for kernel-col j (wrap at W)."""
    if j == 0:
        return [(0, 1, W - 1), (1, W, 0)]
    if j == 2:
        return [(W - 1, W, 0), (0, W - 1, 1)]
    return [(0, W, 0)]


def _build_module():
    import concourse.bass as bass
    import concourse.mybir as mybir
    import concourse.tile as tile

    _patch_tile_drain()

    f32 = mybir.dt.float32
    bf16 = mybir.dt.bfloat16

    nc = bass.Bass()
    x_d = nc.dram_tensor("input", [BPC, C, H, W], bf16, kind="ExternalInput")
    # weights pre-transposed host-side: wts[64*b+c, tap, o]
    w_d = nc.dram_tensor("wts", [128, KH * KW, O], bf16, kind="ExternalInput")
    b_d = nc.dram_tensor("bias", [BPC, O], f32, kind="ExternalInput")
    y_d = nc.dram_tensor("out", [BPC, O, H, W], bf16, kind="ExternalOutput")

    with tile.TileContext(nc) as tc:
        from contextlib import ExitStack

        ctx = ExitStack()
        with ctx:
            persist = ctx.enter_context(tc.tile_pool(name="persist", bufs=1))
            psum = ctx.enter_context(tc.tile_pool(name="psum", bufs=2, space="PSUM"))
            ostage = ctx.enter_context(tc.tile_pool(name="ostage", bufs=2))

            # --- loads. dma_start costs ~650ns of issuing-queue time, so the
            # two HWDGE queues (sync + scalar) issue in parallel with the
            # group-0 criticals first: rows 0-16 on sync, weights on scalar.
            # Full 128-partition DMAs keep all 16 engines at full rate.
            raw = persist.tile([128, H, W], bf16)
            x_bc = x_d.rearrange("b c h w -> (b c) h w")
            wts = persist.tile([128, KH * KW, O], bf16)
            bias_sb = persist.tile([128, 1], f32)

            nc.sync.dma_start(out=raw[:, 0:17, :], in_=x_bc[:, 0:17, :])
            nc.scalar.dma_start(out=wts, in_=w_d[:, :, :])
            nc.sync.dma_start(out=raw[:, H - 1 :, :], in_=x_bc[:, H - 1 :, :])
            nc.sync.dma_start(
                out=bias_sb,
                in_=b_d.rearrange("b o -> (b o)").rearrange("(p x) -> p x", x=1),
            )
            # ACT act-table preload (first use costs ~1.3us; hide in load phase)
            act_warm = persist.tile([128, 1], f32)
            nc.scalar.activation(
                out=act_warm,
                in_=bias_sb,
                func=mybir.ActivationFunctionType.Identity,
                bias=bias_sb,
            )
            # bulk image loads behind the criticals, small-to-large
            for r0, nr in [(17, 24), (41, 32), (73, H - 1 - 73)]:
                nc.scalar.dma_start(
                    out=raw[:, r0 : r0 + nr, :], in_=x_bc[:, r0 : r0 + nr, :]
                )

            y_bo = y_d.rearrange("b o h w -> (b o) h w")

            # --- main loop: groups of G spatial tiles, tap loop outermost
            # within a group so LDWEIGHTS amortizes over G matmuls.
            for t0 in range(0, N_TILES, G):
                tiles = list(range(t0, min(t0 + G, N_TILES)))
                ps = {
                    t: psum.tile([128, BPC, QROWS, W], f32, name=f"ps{t - t0}")
                    for t in tiles
                }

                # chains[(t, s, g)] -> list of (lhsT, out_ap, rhs_ap), in
                # tap-pass order (pieces of one tap stay consecutive)
                chains = {}
                for t in tiles:
                    for s in range(BPC):
                        for g in range(2):
                            mms = []
                            for i, j in TAPS:
                                lhsT = wts[64 * s : 64 * s + 64, i * KW + j, :]
                                r0 = t * TILE_ROWS + QROWS * g + i - 1
                                for ro0, ro1, ir in _row_pieces(r0):
                                    nr = ro1 - ro0
                                    for co0, co1, ic in _col_pieces(j):
                                        ncol = co1 - co0
                                        out = ps[t][
                                            64 * g : 64 * g + 64, s, ro0:ro1, co0:co1
                                        ]
                                        rhs = raw[
                                            64 * s : 64 * s + 64,
                                            ir : ir + nr,
                                            ic : ic + ncol,
                                        ]
                                        mms.append((lhsT, out, rhs))
                            chains[(t, s, g)] = mms

                emitted = {k: 0 for k in chains}
                for p, (i, j) in enumerate(TAPS):
                    for t in tiles:
                        for s in range(BPC):
                            for g in range(2):
                                r0 = t * TILE_ROWS + QROWS * g + i - 1
                                npieces = len(_row_pieces(r0)) * len(_col_pieces(j))
                                mms = chains[(t, s, g)]
                                k0 = emitted[(t, s, g)]
                                for k in range(k0, k0 + npieces):
                                    lhsT, out, rhs = mms[k]
                                    nc.tensor.matmul(
                                        out,
                                        lhsT=lhsT,
                                        rhs=rhs,
                                        start=(k == 0),
                                        stop=(k == len(mms) - 1),
                                    )
                                emitted[(t, s, g)] = k0 + npieces

                # --- evacuate each finished tile: per sample, the g=s
                # quadrant is partition-aligned with the output slot (DVE
                # +bias), the other quadrant crosses partitions (ACT +bias).
                for t in tiles:
                    g4 = t % OGROUP
                    if g4 == 0:
                        st = ostage.tile([128, OGROUP * TILE_ROWS, W], bf16)
                    row0 = g4 * TILE_ROWS
                    for s in range(BPC):
                        home = slice(64 * s, 64 * s + 64)
                        away = slice(64 - 64 * s, 128 - 64 * s)
                        nc.vector.tensor_scalar_add(
                            st[home, row0 + QROWS * s : row0 + QROWS * s + QROWS, :],
                            ps[t][home, s, :, :],
                            bias_sb[home, :],
                        )
                        nc.scalar.activation(
                            out=st[
                                home,
                                row0 + QROWS * (1 - s) : row0 + QROWS * (2 - s),
                                :,
                            ],
                            in_=ps[t][away, s, :, :],
                            func=mybir.ActivationFunctionType.Identity,
                            bias=bias_sb[home, :],
                        )

                    if t == N_TILES - 3:
                        # flush the first half of the last group early so the
                        # end-of-kernel store tail is only 16 rows
                        nc.sync.dma_start(out=y_bo[:, 96:112, :], in_=st[:, 0:16, :])
                    if g4 == OGROUP - 1:
                        gr0 = (t - OGROUP + 1) * TILE_ROWS
                        if t == N_TILES - 1:
                            nc.sync.dma_start(
                                out=y_bo[:, 112:128, :], in_=st[:, 16:32, :]
                            )
                        else:
                            nc.sync.dma_start(
                                out=y_bo[:, gr0 : gr0 + OGROUP * TILE_ROWS, :], in_=st
                            )
    return nc


def _get_module():
    # NOTE: walrus's --enable-ldw-opt crashes codegen (visitInstLdweights),
    # so every matmul pays its serial 64-column LDWEIGHTS (~53ns).
    if "nc" not in _CACHE:
        nc = _build_module()
        _split_multi_waits(nc)
        _CACHE["nc"] = nc
    return _CACHE["nc"]


def _in_maps(input, weight, bias):
    import ml_dtypes

    bf16 = ml_dtypes.bfloat16
    maps = []
    for i in range(N_CORES):
        lo, hi = i * BPC, (i + 1) * BPC
        # wts[64b+c, tap, o] = w[b, c*9+tap, o]
        wloc = weight[lo:hi].reshape(BPC, C, KH * KW, O)
        maps.append(
            {
                "input": np.ascontiguousarray(input[lo:hi]).astype(bf16),
                "wts": np.ascontiguousarray(wloc.reshape(BPC * C, KH * KW, O)).astype(
                    bf16
                ),
                "bias": np.ascontiguousarray(bias[lo:hi]),
            }
        )
    return maps


def kernel(input, weight, bias):
    from concourse.bass_utils import run_bass_kernel_spmd

    nc = _get_module()
    res = run_bass_kernel_spmd(
        nc, _in_maps(input, weight, bias), core_ids=list(range(N_CORES))
    )
    return np.concatenate(
        [res.results[i]["out"] for i in range(N_CORES)], axis=0
    ).astype(np.float32)
